# revision 13
# baseline (speedup 1.0000x reference)
"""Trainium2 Bass kernel for Luong-style attention.

Reference computation (per full problem):
    h = decoder_hidden @ W.T + b          # [B, De]
    enc = encoder_output.transpose(1,0,2) # [B, S, De]
    a = softmax(einsum('bsd,bd->bs', enc, h), axis=1)
    context = einsum('bs,bsd->bd', a, enc)  # [B, De]

Shapes: B=64, S=4096, Dd=1024, De=512 (f32).

Strategy: data-parallel over B across 8 NeuronCores (B_local=8 each).
encoder_output is the huge tensor (512 MB); each core streams its
64 MB shard from HBM exactly once (chunked two-level softmax).
Per 128-row s-tile (f32, no bf16 copy of the stream):
  - scores via DVE scalar_tensor_tensor (product + row-sum in one op)
    against a partition-broadcast fp16 copy of h (one 2-byte source
    keeps the DVE at full rate; two f32 sources would halve it),
  - per-chunk softmax via PE transpose + ACT exp (bias=-chunk_max,
    fused row-sum),
  - context accumulated in a single [8, 512] PSUM bank: for each b,
    matmul(lhsT=prob column [128,1] fp32r, rhs=enc f32r [128,512]) ->
    out row [1,512] at partition b.  fp32r moving data runs at
    1 cycle/row for N>=256, and a 1-column weight load is ~free, so
    the PE streams the f32 tile directly (no bf16 cast needed).
    The accumulation group is opened by one bank-wide zero matmul
    (start=True clears has_written bits bank-wide).
  - chunks combined at the end with exp(m_c - M)/l weights; partials
    are already in natural [b, d] layout so the combine is a short
    chain of [8,512] DVE ops.
No collectives needed.  Bottleneck is the HBM stream (~187 us for
64 MB/core at ~358 GB/s); all engines sit below that.
"""

import numpy as np

import concourse.bass as bass
import concourse.bacc as bacc_mod
import concourse.tile as tile
import concourse.mybir as mybir
from concourse import masks
from concourse.bass_utils import run_bass_kernel_spmd

F32 = mybir.dt.float32
F32R = mybir.dt.float32r
F16 = mybir.dt.float16
BF16 = mybir.dt.bfloat16
ALU = mybir.AluOpType
ACTF = mybir.ActivationFunctionType
AX = mybir.AxisListType

NCORES = 8
B = 8          # per-core batch
S = 4096
DD = 1024
DE = 512
P = 128        # s-values per tile
NTILES = S // P          # 32
CHUNK_TILES = 4          # s-tiles per softmax chunk
NCHUNK = NTILES // CHUNK_TILES   # 8


def build_nc(ntiles: int = NTILES):
    nchunk = ntiles // CHUNK_TILES
    s_local = ntiles * P
    nc = bacc_mod.Bacc("TRN2", target_bir_lowering=False, debug=False)
    dec_d = nc.dram_tensor("decoder_hidden", [B, DD], F32, kind="ExternalInput")
    enc_d = nc.dram_tensor("encoder_output", [s_local, B, DE], F32, kind="ExternalInput")
    w_d = nc.dram_tensor("W", [DE, DD], F32, kind="ExternalInput")
    b_d = nc.dram_tensor("b", [DE], F32, kind="ExternalInput")
    out_d = nc.dram_tensor("out", [B, DE], F32, kind="ExternalOutput")

    with tile.TileContext(nc) as tc:
        with (
            tc.tile_pool(name="const", bufs=1) as const_pool,
            tc.tile_pool(name="persist", bufs=1) as persist_pool,
            tc.tile_pool(name="enc", bufs=5) as enc_pool,
            tc.tile_pool(name="junk", bufs=6) as junk_pool,
            tc.tile_pool(name="scores", bufs=6) as sc_pool,
            tc.tile_pool(name="probs", bufs=2) as p_pool,
            tc.tile_pool(name="pt", bufs=4) as pt_pool,
        ):
            wload_cm = tc.tile_pool(name="wload", bufs=2)
            wload_pool = wload_cm.__enter__()
            wt_cm = tc.tile_pool(name="wt", bufs=1)
            wt_pool = wt_cm.__enter__()
            setup_psum_cm = tc.tile_pool(name="psum_setup", bufs=4, space="PSUM")
            psum_setup = setup_psum_cm.__enter__()
            setup_psum2_cm = tc.tile_pool(name="psum_setup2", bufs=4, space="PSUM")
            psum_setup2 = setup_psum2_cm.__enter__()
            # ---- constants ----
            ident = const_pool.tile([P, P], F32)
            masks.make_identity(nc, ident[:])
            ones = const_pool.tile([1, P], F32)
            nc.vector.memset(ones[:], 1.0)
            # row-broadcast selectors: sel[:, bb, :] is [8, 128] with row bb
            # all-ones; matmul(sel_bb, x) broadcasts x's row bb to all
            # 128 partitions without any cross-partition DMA.
            sel = const_pool.tile([B, B, P], F32)
            nc.gpsimd.memset(sel[:], 0.0)
            # sel[k, bb, m] = 1.0 iff k == bb  (k*1 + bb*(-1) == 0)
            nc.gpsimd.affine_select(
                out=sel[:], in_=sel[:],
                compare_op=ALU.not_equal, fill=1.0, base=0,
                pattern=[[-1, B], [0, P]], channel_multiplier=1)

            # ---- load small inputs ----
            dec_sb = const_pool.tile([B, DD], F32)
            nc.sync.dma_start(dec_sb[:], dec_d[:])
            bias_sb = const_pool.tile([1, DE], F32)
            nc.sync.dma_start(bias_sb[:], b_d[None, :])
            setup_dmas = []

            # ---- transpose dec: [8,1024] -> decT [128, 8, 8] (chunk c = cols c*128..) ----
            decT = const_pool.tile([P, DD // P, B], F32)
            for c in range(DD // P):
                tp = psum_setup.tile([P, B], F32, tag="su")
                nc.tensor.transpose(tp[:], dec_sb[:, c * P:(c + 1) * P], ident[0:B, 0:B])
                nc.vector.tensor_copy(decT[:, c, :], tp[:])

            # ---- transpose W: [512,1024] -> WT [128, 8, 512] (chunk c = W.T rows c*128..) ----
            wt_sb = wt_pool.tile([P, DD // P, DE], F32)
            for wi in range(DE // P):
                w_row = wload_pool.tile([P, DD], F32, tag="wrow")
                half = DD // 2
                setup_dmas.append(nc.sync.dma_start(
                    w_row[:, 0:half], w_d[wi * P:(wi + 1) * P, 0:half]))
                setup_dmas.append(nc.sync.dma_start(
                    w_row[:, half:DD], w_d[wi * P:(wi + 1) * P, half:DD]))
                for c in range(DD // P):
                    tp = psum_setup.tile([P, P], F32, tag="su")
                    nc.tensor.transpose(tp[:], w_row[:, c * P:(c + 1) * P], ident[:])
                    nc.vector.tensor_copy(wt_sb[:, c, wi * P:(wi + 1) * P], tp[:])

            # ---- h = dec @ W.T + b  -> h_sb [8, 512] ----
            h_ps = psum_setup2.tile([B, DE], F32, tag="hsu")
            for c in range(DD // P):
                nc.tensor.matmul(h_ps[:], decT[:, c, :], wt_sb[:, c, :],
                                 start=(c == 0), stop=False)
            nc.tensor.matmul(h_ps[:], ones[0:1, 0:B], bias_sb[:],
                             start=False, stop=True)
            h_sb = const_pool.tile([B, DE], F32)
            nc.vector.tensor_copy(h_sb[:], h_ps[:])
            # reciprocal of the fp16-rounded h (same rounding as hb below);
            # the context partials carry a factor h_q that this divides out.
            hq = const_pool.tile([B, DE], F16)
            nc.scalar.copy(hq[:], h_sb[:])
            h_rcp = const_pool.tile([B, DE], F32)
            nc.vector.reciprocal(h_rcp[:], hq[:])

            # ---- broadcast h along partitions: hb [128, 8, 512] fp16 ----
            # selector matmul: out = sel_bb.T @ h_sb puts h row bb on all
            # 128 partitions; the psum->sbuf copy converts to fp16 so the
            # score stt has only one 4-byte source (full DVE rate).
            hb = persist_pool.tile([P, B, DE], F16)
            for bb in range(B):
                hp = psum_setup2.tile([P, DE], F32, tag="hsu")
                nc.tensor.matmul(hp[:], sel[:, bb, :], h_sb[:],
                                 start=True, stop=True)
                nc.scalar.copy(hb[:, bb, :], hp[:])

            setup_psum2_cm.__exit__(None, None, None)
            setup_psum_cm.__exit__(None, None, None)
            wt_cm.__exit__(None, None, None)
            wload_cm.__exit__(None, None, None)
            _tr_cm = tc.tile_pool(name="psum_tr", bufs=2, space="PSUM")
            psum_tr = _tr_cm.__enter__()
            _sc_cm = tc.tile_pool(name="psum_sc", bufs=2, space="PSUM")
            psum_sc = _sc_cm.__enter__()
            _ctx_cm = tc.tile_pool(name="psum_ctx", bufs=2, space="PSUM")
            psum_ctx = _ctx_cm.__enter__()

            # ---- diagonal prob-weight tiles ----
            # pz[p, b, col] = prob_col_b[p] iff col == b else 0.  pz[:, b, :]
            # is a [128, 128] bf16 weight (FWL-eligible, ~44ns load) whose
            # single nonzero column routes batch b's context row to psum
            # partition b; the other 127 output rows accumulate zeros.
            # Off-diagonals are zeroed once here and never written again;
            # each tile only refreshes the 8 diagonal slots (stride-129 AP).
            NPZ = 4
            pzs = [persist_pool.tile([P, B, P], BF16, name=f"pz{i}")
                   for i in range(NPZ)]
            pz_diags = []
            for pz in pzs:
                nc.vector.memset(pz[:], 0.0)
                pz_diags.append(pz[:].rearrange("p a b -> p (a b)")[:, 0:B * P:P + 1])

            # ---- online-softmax running state (flash-attention style) ----
            # Chunks of 4 tiles, then 1-tile mini-chunks at the end so the
            # post-stream tail is one small softmax + 8 matmuls, not a whole
            # chunk + combine chain.
            chunk_sizes = [CHUNK_TILES] * (ntiles // CHUNK_TILES - 1) + [1] * CHUNK_TILES
            Mr = [persist_pool.tile([B, 1], F32, name=f"Mrun{i}") for i in range(2)]
            lr = [persist_pool.tile([B, 1], F32, name=f"lrun{i}") for i in range(2)]
            acc = [persist_pool.tile([B, DE], F32, name=f"acc{i}") for i in range(2)]

            # ---- main streaming loop over S ----
            j = 0
            for c, ct in enumerate(chunk_sizes):
                prod_tiles = []
                scT = psum_sc.tile([B, CHUNK_TILES * P], F32)
                for t in range(ct):
                    et = enc_pool.tile([P, B, DE], F32)
                    enc_dma = nc.sync.dma_start(et[:], enc_d[(j + t) * P:(j + t + 1) * P, :, :])
                    if j + t == 0:
                        for sd in setup_dmas:
                            tile.add_dep_helper(enc_dma.ins, sd.ins,
                                                reason="let setup W loads win HBM first")
                    # products enc*h_q in bf16; the row-sum accumulator
                    # yields the scores, and the product tile itself is the
                    # context matmul's moving operand (ctx = sum p*prod / h_q),
                    # so the stream never needs a separate bf16 cast.
                    jt = junk_pool.tile([P, B, DE], BF16, tag="junk")
                    prod_tiles.append(jt)
                    sct = sc_pool.tile([P, B], F32)
                    for bb in range(B):
                        nc.vector.scalar_tensor_tensor(
                            out=jt[:, bb, :],
                            in0=et[:, bb, :],
                            scalar=1.0,
                            in1=hb[:, bb, :],
                            op0=ALU.mult,
                            op1=ALU.mult,
                            accum_out=sct[:, bb:bb + 1],
                        )
                    # transpose scores into [8, 128] slice of chunk psum
                    nc.tensor.transpose(scT[:, t * P:(t + 1) * P], sct[:], ident[:])

                # chunk softmax against the RUNNING max (flash style): the
                # chunk's probs are already scaled by exp(m - M_new), so the
                # psum partial needs no per-chunk reweighting later; only the
                # accumulator gets rescaled by alpha = exp(M_old - M_new).
                m_c = sc_pool.tile([B, 1], F32, tag="stat")
                nc.vector.reduce_max(m_c[:], scT[:, 0:ct * P], axis=AX.X)
                negm = sc_pool.tile([B, 1], F32, tag="stat")
                l_c = sc_pool.tile([B, 1], F32, tag="stat")
                if c == 0:
                    nc.vector.tensor_copy(Mr[0][:], m_c[:])
                else:
                    nc.vector.tensor_tensor(out=Mr[c % 2][:], in0=Mr[(c - 1) % 2][:],
                                            in1=m_c[:], op=ALU.max)
                nc.vector.tensor_scalar_mul(negm[:], Mr[c % 2][:], -1.0)
                p_sb = p_pool.tile([B, CHUNK_TILES * P], F32)
                nc.scalar.activation(p_sb[:, 0:ct * P], scT[:, 0:ct * P], ACTF.Exp,
                                     bias=negm[:], scale=1.0,
                                     accum_out=l_c[:])

                # context partial: ctx[b, :] += sum_s p[s, b] * prod[s, b, :]
                # in ONE [128, 512] psum bank; the first matmul's start=True
                # clears the bank (its 127 zero weight columns write zeros).
                ctx_ps = psum_ctx.tile([P, DE], F32)
                for t in range(ct):
                    ptp = psum_tr.tile([P, B], F32, tag="tr")
                    nc.tensor.transpose(ptp[:], p_sb[:, t * P:(t + 1) * P], ident[0:B, 0:B])
                    pz, pzd = pzs[(j + t) % NPZ], pz_diags[(j + t) % NPZ]
                    nc.scalar.copy(pzd, ptp[:])
                    for bb in range(B):
                        nc.tensor.matmul(
                            ctx_ps[:],
                            pz[:, bb, :],
                            prod_tiles[t][:, bb, :],
                            start=(t == 0 and bb == 0),
                            stop=(t == ct - 1 and bb == B - 1))

                # fold the chunk into the running accumulator
                if c == 0:
                    nc.scalar.copy(acc[0][:], ctx_ps[0:B, :])
                    nc.vector.tensor_copy(lr[0][:], l_c[:])
                else:
                    alpha = sc_pool.tile([B, 1], F32, tag="stat")
                    nc.scalar.activation(alpha[:], Mr[(c - 1) % 2][:], ACTF.Exp,
                                         bias=negm[:], scale=1.0)
                    nc.vector.scalar_tensor_tensor(
                        out=lr[c % 2][:], in0=lr[(c - 1) % 2][:], scalar=alpha[:, 0:1],
                        in1=l_c[:], op0=ALU.mult, op1=ALU.add)
                    nc.vector.scalar_tensor_tensor(
                        out=acc[c % 2][:], in0=acc[(c - 1) % 2][:], scalar=alpha[:, 0:1],
                        in1=ctx_ps[0:B, :], op0=ALU.mult, op1=ALU.add)
                j += ct

            # ---- finalize: divide by l_total and by h_q, store ----
            nchunks_total = len(chunk_sizes)
            last = (nchunks_total - 1) % 2
            g_rl = persist_pool.tile([B, 1], F32)
            nc.vector.reciprocal(g_rl[:], lr[last][:])
            final_sb = persist_pool.tile([B, DE], F32)
            nc.vector.scalar_tensor_tensor(
                out=final_sb[:], in0=acc[last][:], scalar=g_rl[:, 0:1],
                in1=h_rcp[:], op0=ALU.mult, op1=ALU.mult)
            nc.sync.dma_start(out_d[:], final_sb[:])
            _ctx_cm.__exit__(None, None, None)
            _sc_cm.__exit__(None, None, None)
            _tr_cm.__exit__(None, None, None)

    nc.compile()
    if not nc.is_finalized():
        nc.finalize()
    return nc


_NC = None


def kernel(decoder_hidden, encoder_output, W, b):
    global _NC
    if _NC is None:
        _NC = build_nc()
    decoder_hidden = np.ascontiguousarray(decoder_hidden, dtype=np.float32)
    encoder_output = np.ascontiguousarray(encoder_output, dtype=np.float32)
    W = np.ascontiguousarray(W, dtype=np.float32)
    b = np.ascontiguousarray(b, dtype=np.float32)

    in_maps = []
    for i in range(NCORES):
        sl = slice(i * B, (i + 1) * B)
        in_maps.append({
            "decoder_hidden": decoder_hidden[sl],
            "encoder_output": np.ascontiguousarray(encoder_output[:, sl, :]),
            "W": W,
            "b": b,
        })
    res = run_bass_kernel_spmd(_NC, in_maps, core_ids=list(range(NCORES)))
    return np.concatenate([res.results[i]["out"] for i in range(NCORES)], axis=0)


# revision 14
# speedup vs baseline: 1.1772x; 1.1772x over previous
"""Trainium2 Bass kernel for Luong-style attention.

Reference computation (per full problem):
    h = decoder_hidden @ W.T + b          # [B, De]
    enc = encoder_output.transpose(1,0,2) # [B, S, De]
    a = softmax(einsum('bsd,bd->bs', enc, h), axis=1)
    context = einsum('bs,bsd->bd', a, enc)  # [B, De]

Shapes: B=64, S=4096, Dd=1024, De=512 (f32).

Strategy: data-parallel over B across 8 NeuronCores (B_local=8 each).
encoder_output is the huge tensor (512 MB); each core streams its
64 MB shard from HBM exactly once (chunked two-level softmax).
Per 128-row s-tile (f32, no bf16 copy of the stream):
  - scores via DVE scalar_tensor_tensor (product + row-sum in one op)
    against a partition-broadcast fp16 copy of h (one 2-byte source
    keeps the DVE at full rate; two f32 sources would halve it),
  - per-chunk softmax via PE transpose + ACT exp (bias=-chunk_max,
    fused row-sum),
  - context accumulated in a single [8, 512] PSUM bank: for each b,
    matmul(lhsT=prob column [128,1] fp32r, rhs=enc f32r [128,512]) ->
    out row [1,512] at partition b.  fp32r moving data runs at
    1 cycle/row for N>=256, and a 1-column weight load is ~free, so
    the PE streams the f32 tile directly (no bf16 cast needed).
    The accumulation group is opened by one bank-wide zero matmul
    (start=True clears has_written bits bank-wide).
  - chunks combined at the end with exp(m_c - M)/l weights; partials
    are already in natural [b, d] layout so the combine is a short
    chain of [8,512] DVE ops.
No collectives needed.  Bottleneck is the HBM stream (~187 us for
64 MB/core at ~358 GB/s); all engines sit below that.
"""

import numpy as np

import concourse.bass as bass
import concourse.bacc as bacc_mod
import concourse.tile as tile
import concourse.mybir as mybir
from concourse import masks
from concourse.bass_utils import run_bass_kernel_spmd

F32 = mybir.dt.float32
F32R = mybir.dt.float32r
F16 = mybir.dt.float16
BF16 = mybir.dt.bfloat16
ALU = mybir.AluOpType
ACTF = mybir.ActivationFunctionType
AX = mybir.AxisListType

NCORES = 8
B = 8          # per-core batch
S = 4096
DD = 1024
DE = 512
P = 128        # s-values per tile
NTILES = S // P          # 32
CHUNK_TILES = 4          # s-tiles per softmax chunk
NCHUNK = NTILES // CHUNK_TILES   # 8


def build_nc(ntiles: int = NTILES):
    nchunk = ntiles // CHUNK_TILES
    s_local = ntiles * P
    nc = bacc_mod.Bacc("TRN2", target_bir_lowering=False, debug=False)
    dec_d = nc.dram_tensor("decoder_hidden", [B, DD], F32, kind="ExternalInput")
    enc_d = nc.dram_tensor("encoder_output", [s_local, B, DE], F32, kind="ExternalInput")
    w_d = nc.dram_tensor("W", [DE, DD], F32, kind="ExternalInput")
    b_d = nc.dram_tensor("b", [DE], F32, kind="ExternalInput")
    out_d = nc.dram_tensor("out", [B, DE], F32, kind="ExternalOutput")

    with tile.TileContext(nc) as tc:
        with (
            tc.tile_pool(name="const", bufs=1) as const_pool,
            tc.tile_pool(name="persist", bufs=1) as persist_pool,
            tc.tile_pool(name="enc", bufs=5) as enc_pool,
            tc.tile_pool(name="junk", bufs=6) as junk_pool,
            tc.tile_pool(name="scores", bufs=6) as sc_pool,
            tc.tile_pool(name="probs", bufs=2) as p_pool,
            tc.tile_pool(name="pt", bufs=4) as pt_pool,
        ):
            wload_cm = tc.tile_pool(name="wload", bufs=2)
            wload_pool = wload_cm.__enter__()
            wt_cm = tc.tile_pool(name="wt", bufs=1)
            wt_pool = wt_cm.__enter__()
            setup_psum_cm = tc.tile_pool(name="psum_setup", bufs=4, space="PSUM")
            psum_setup = setup_psum_cm.__enter__()
            setup_psum2_cm = tc.tile_pool(name="psum_setup2", bufs=4, space="PSUM")
            psum_setup2 = setup_psum2_cm.__enter__()
            # ---- constants ----
            ident = const_pool.tile([P, P], F32)
            masks.make_identity(nc, ident[:])
            ones = const_pool.tile([1, P], F32)
            nc.vector.memset(ones[:], 1.0)
            # row-broadcast selectors: sel[:, bb, :] is [8, 128] with row bb
            # all-ones; matmul(sel_bb, x) broadcasts x's row bb to all
            # 128 partitions without any cross-partition DMA.
            sel = const_pool.tile([B, B, P], F32)
            nc.gpsimd.memset(sel[:], 0.0)
            # sel[k, bb, m] = 1.0 iff k == bb  (k*1 + bb*(-1) == 0)
            nc.gpsimd.affine_select(
                out=sel[:], in_=sel[:],
                compare_op=ALU.not_equal, fill=1.0, base=0,
                pattern=[[-1, B], [0, P]], channel_multiplier=1)

            # ---- load small inputs ----
            dec_sb = const_pool.tile([B, DD], F32)
            nc.sync.dma_start(dec_sb[:], dec_d[:])
            bias_sb = const_pool.tile([1, DE], F32)
            nc.sync.dma_start(bias_sb[:], b_d[None, :])
            setup_dmas = []

            # ---- transpose dec: [8,1024] -> decT [128, 8, 8] (chunk c = cols c*128..) ----
            decT = const_pool.tile([P, DD // P, B], F32)
            for c in range(DD // P):
                tp = psum_setup.tile([P, B], F32, tag="su")
                nc.tensor.transpose(tp[:], dec_sb[:, c * P:(c + 1) * P], ident[0:B, 0:B])
                nc.vector.tensor_copy(decT[:, c, :], tp[:])

            # ---- transpose W: [512,1024] -> WT [128, 8, 512] (chunk c = W.T rows c*128..) ----
            wt_sb = wt_pool.tile([P, DD // P, DE], F32)
            for wi in range(DE // P):
                w_row = wload_pool.tile([P, DD], F32, tag="wrow")
                half = DD // 2
                setup_dmas.append(nc.sync.dma_start(
                    w_row[:, 0:half], w_d[wi * P:(wi + 1) * P, 0:half]))
                setup_dmas.append(nc.sync.dma_start(
                    w_row[:, half:DD], w_d[wi * P:(wi + 1) * P, half:DD]))
                for c in range(DD // P):
                    tp = psum_setup.tile([P, P], F32, tag="su")
                    nc.tensor.transpose(tp[:], w_row[:, c * P:(c + 1) * P], ident[:])
                    nc.vector.tensor_copy(wt_sb[:, c, wi * P:(wi + 1) * P], tp[:])

            # ---- h = dec @ W.T + b  -> h_sb [8, 512] ----
            h_ps = psum_setup2.tile([B, DE], F32, tag="hsu")
            for c in range(DD // P):
                nc.tensor.matmul(h_ps[:], decT[:, c, :], wt_sb[:, c, :],
                                 start=(c == 0), stop=False)
            nc.tensor.matmul(h_ps[:], ones[0:1, 0:B], bias_sb[:],
                             start=False, stop=True)
            h_sb = const_pool.tile([B, DE], F32)
            nc.vector.tensor_copy(h_sb[:], h_ps[:])
            # reciprocal of the fp16-rounded h (same rounding as hb below);
            # the context partials carry a factor h_q that this divides out.
            hq = const_pool.tile([B, DE], F16)
            nc.scalar.copy(hq[:], h_sb[:])
            h_rcp = const_pool.tile([B, DE], F32)
            nc.vector.reciprocal(h_rcp[:], hq[:])

            # ---- broadcast h along partitions: hb [128, 8, 512] fp16 ----
            # selector matmul: out = sel_bb.T @ h_sb puts h row bb on all
            # 128 partitions; the psum->sbuf copy converts to fp16 so the
            # score stt has only one 4-byte source (full DVE rate).
            hb = persist_pool.tile([P, B, DE], F16)
            for bb in range(B):
                hp = psum_setup2.tile([P, DE], F32, tag="hsu")
                nc.tensor.matmul(hp[:], sel[:, bb, :], h_sb[:],
                                 start=True, stop=True)
                nc.scalar.copy(hb[:, bb, :], hp[:])

            setup_psum2_cm.__exit__(None, None, None)
            setup_psum_cm.__exit__(None, None, None)
            wt_cm.__exit__(None, None, None)
            wload_cm.__exit__(None, None, None)
            _tr_cm = tc.tile_pool(name="psum_tr", bufs=2, space="PSUM")
            psum_tr = _tr_cm.__enter__()
            _sc_cm = tc.tile_pool(name="psum_sc", bufs=2, space="PSUM")
            psum_sc = _sc_cm.__enter__()
            _ctx_cm = tc.tile_pool(name="psum_ctx", bufs=2, space="PSUM")
            psum_ctx = _ctx_cm.__enter__()

            # ---- diagonal prob-weight tiles ----
            # pz[p, b, b'] = prob_col_b[p] iff b' == b else 0.  pz[:, b, :]
            # is a [128, 8] bf16 weight whose single nonzero column routes
            # batch b's context row to psum partition b while keeping the
            # matmul base partition at 0.  Off-diagonals are zeroed once here
            # and never written again; each tile only refreshes the 8
            # diagonal slots (stride-9 AP).
            NPZ = 4
            pzs = [persist_pool.tile([P, B, B], BF16, name=f"pz{i}")
                   for i in range(NPZ)]
            pz_diags = []
            for pz in pzs:
                nc.vector.memset(pz[:], 0.0)
                pz_diags.append(pz[:].rearrange("p a b -> p (a b)")[:, 0:B * B:B + 1])

            # ---- online-softmax running state (flash-attention style) ----
            # Chunks of 4 tiles, then 1-tile mini-chunks at the end so the
            # post-stream tail is one small softmax + 8 matmuls, not a whole
            # chunk + combine chain.
            chunk_sizes = [CHUNK_TILES] * (ntiles // CHUNK_TILES - 1) + [1] * CHUNK_TILES
            Mr = [persist_pool.tile([B, 1], F32, name=f"Mrun{i}") for i in range(2)]
            lr = [persist_pool.tile([B, 1], F32, name=f"lrun{i}") for i in range(2)]
            acc = [persist_pool.tile([B, DE], F32, name=f"acc{i}") for i in range(2)]

            # ---- main streaming loop over S ----
            j = 0
            for c, ct in enumerate(chunk_sizes):
                prod_tiles = []
                scT = psum_sc.tile([B, CHUNK_TILES * P], F32)
                for t in range(ct):
                    et = enc_pool.tile([P, B, DE], F32)
                    enc_dma = nc.sync.dma_start(et[:], enc_d[(j + t) * P:(j + t + 1) * P, :, :])
                    if j + t == 0:
                        for sd in setup_dmas:
                            tile.add_dep_helper(enc_dma.ins, sd.ins,
                                                reason="let setup W loads win HBM first")
                    # products enc*h_q in bf16; the row-sum accumulator
                    # yields the scores, and the product tile itself is the
                    # context matmul's moving operand (ctx = sum p*prod / h_q),
                    # so the stream never needs a separate bf16 cast.
                    jt = junk_pool.tile([P, B, DE], BF16, tag="junk")
                    prod_tiles.append(jt)
                    sct = sc_pool.tile([P, B], F32)
                    for bb in range(B):
                        nc.vector.scalar_tensor_tensor(
                            out=jt[:, bb, :],
                            in0=et[:, bb, :],
                            scalar=1.0,
                            in1=hb[:, bb, :],
                            op0=ALU.mult,
                            op1=ALU.mult,
                            accum_out=sct[:, bb:bb + 1],
                        )
                    # transpose scores into [8, 128] slice of chunk psum
                    nc.tensor.transpose(scT[:, t * P:(t + 1) * P], sct[:], ident[:])

                # chunk softmax against the RUNNING max (flash style): the
                # chunk's probs are already scaled by exp(m - M_new), so the
                # psum partial needs no per-chunk reweighting later; only the
                # accumulator gets rescaled by alpha = exp(M_old - M_new).
                m_c = sc_pool.tile([B, 1], F32, tag="stat")
                nc.vector.reduce_max(m_c[:], scT[:, 0:ct * P], axis=AX.X)
                negm = sc_pool.tile([B, 1], F32, tag="stat")
                l_c = sc_pool.tile([B, 1], F32, tag="stat")
                if c == 0:
                    nc.vector.tensor_copy(Mr[0][:], m_c[:])
                else:
                    nc.vector.tensor_tensor(out=Mr[c % 2][:], in0=Mr[(c - 1) % 2][:],
                                            in1=m_c[:], op=ALU.max)
                nc.vector.tensor_scalar_mul(negm[:], Mr[c % 2][:], -1.0)
                p_sb = p_pool.tile([B, CHUNK_TILES * P], F32)
                nc.scalar.activation(p_sb[:, 0:ct * P], scT[:, 0:ct * P], ACTF.Exp,
                                     bias=negm[:], scale=1.0,
                                     accum_out=l_c[:])

                # context partial: ctx[b, :] += sum_s p[s, b] * prod[s, b, :]
                # in ONE [8, 512] psum bank; the first matmul's start=True
                # clears the bank (its 7 zero weight columns write zeros).
                ctx_ps = psum_ctx.tile([B, DE], F32)
                for t in range(ct):
                    ptp = psum_tr.tile([P, B], F32, tag="tr")
                    nc.tensor.transpose(ptp[:], p_sb[:, t * P:(t + 1) * P], ident[0:B, 0:B])
                    pz, pzd = pzs[(j + t) % NPZ], pz_diags[(j + t) % NPZ]
                    nc.scalar.copy(pzd, ptp[:])
                    for bb in range(B):
                        nc.tensor.matmul(
                            ctx_ps[:],
                            pz[:, bb, :],
                            prod_tiles[t][:, bb, :],
                            start=(t == 0 and bb == 0),
                            stop=(t == ct - 1 and bb == B - 1))

                # fold the chunk into the running accumulator
                if c == 0:
                    nc.scalar.copy(acc[0][:], ctx_ps[:])
                    nc.vector.tensor_copy(lr[0][:], l_c[:])
                else:
                    alpha = sc_pool.tile([B, 1], F32, tag="stat")
                    nc.scalar.activation(alpha[:], Mr[(c - 1) % 2][:], ACTF.Exp,
                                         bias=negm[:], scale=1.0)
                    nc.vector.scalar_tensor_tensor(
                        out=lr[c % 2][:], in0=lr[(c - 1) % 2][:], scalar=alpha[:, 0:1],
                        in1=l_c[:], op0=ALU.mult, op1=ALU.add)
                    nc.vector.scalar_tensor_tensor(
                        out=acc[c % 2][:], in0=acc[(c - 1) % 2][:], scalar=alpha[:, 0:1],
                        in1=ctx_ps[:], op0=ALU.mult, op1=ALU.add)
                j += ct

            # ---- finalize: divide by l_total and by h_q, store ----
            nchunks_total = len(chunk_sizes)
            last = (nchunks_total - 1) % 2
            g_rl = persist_pool.tile([B, 1], F32)
            nc.vector.reciprocal(g_rl[:], lr[last][:])
            final_sb = persist_pool.tile([B, DE], F32)
            nc.vector.scalar_tensor_tensor(
                out=final_sb[:], in0=acc[last][:], scalar=g_rl[:, 0:1],
                in1=h_rcp[:], op0=ALU.mult, op1=ALU.mult)
            nc.sync.dma_start(out_d[:], final_sb[:])
            _ctx_cm.__exit__(None, None, None)
            _sc_cm.__exit__(None, None, None)
            _tr_cm.__exit__(None, None, None)

    nc.compile()
    if not nc.is_finalized():
        nc.finalize()
    return nc


_NC = None


def kernel(decoder_hidden, encoder_output, W, b):
    global _NC
    if _NC is None:
        _NC = build_nc()
    decoder_hidden = np.ascontiguousarray(decoder_hidden, dtype=np.float32)
    encoder_output = np.ascontiguousarray(encoder_output, dtype=np.float32)
    W = np.ascontiguousarray(W, dtype=np.float32)
    b = np.ascontiguousarray(b, dtype=np.float32)

    in_maps = []
    for i in range(NCORES):
        sl = slice(i * B, (i + 1) * B)
        in_maps.append({
            "decoder_hidden": decoder_hidden[sl],
            "encoder_output": np.ascontiguousarray(encoder_output[:, sl, :]),
            "W": W,
            "b": b,
        })
    res = run_bass_kernel_spmd(_NC, in_maps, core_ids=list(range(NCORES)))
    return np.concatenate([res.results[i]["out"] for i in range(NCORES)], axis=0)
